# revision 7
# baseline (speedup 1.0000x reference)
"""Trainium2 Bass kernel v4 for the dense transformer encoder block
(B=4, S=2048, D=1024, H=16, MLP=4096). 8 cores = 4 batch x 2 query-halves,
kv host-reordered so each core's 1024 queries come first.

656,492 ns (TimelineSim) vs the 861,764 ns f32r baseline; rel err 4.7e-3.

- fp8(e4m3) DoubleRow matmuls (0.5 cy/row, 2x128-deep per instr) for the
  attention path (Q/K/V/O projections, scores, AV). x32 weight prescale
  (exact power of 2) keeps e4m3 out of subnormals; the 1/32 factors fold
  into the exp scale (1/8192) and O-proj copy (1/1024). Scores pad dh=64
  via zeroed qpad rows/k-tiles (cost is output-column driven, padding is
  free); the DR tile1 of the K operand points at the next k-tile's data
  (legal because the matching rhs tile is zero).
- AV feature-major: out[64, q] plus a parallel all-ones-lhsT chain giving
  the softmax denominator (dual-fp8 ldweights only allows 64/128-wide
  tiles, and DR psum targets must start at partition 0). Normalize = DVE
  reciprocal + Pool partition_broadcast + one DVE multiply out of PSUM;
  attention output lands feature-major so O-proj needs no transposes.
- exp on ACT: scale 1/8192, bias -3 (e4m3 overflow headroom). The scores
  psum is two half-q-width [128, 2kt, 2h, 256] tiles used alternately so
  score matmuls double-buffer against the exp reads.
- MLP bf16 (fp8 MLP exceeds the 2e-2 budget; measured). W1/W2 streamed in
  double-buffered chunks (64KB/partition each - cannot be resident). Gelu
  deferred and batched in-place (bias added in the psum->bf16 copy) and
  LN2's sqrt batched per slice, so the ACT table rarely reloads.
- 2-slice software pipeline over the 1024 queries: slice s's ACT-bound
  softmax overlaps slice s-1's PE-bound MLP via a chunked generator
  (~1-3us chunks consumed at the exp stall points); K-projections are
  slice-invariant, hoisted, and streamed into slice 0's stalls.
"""

import os
import sys

sys.path.insert(0, "/opt/trn_rl_repo")

from contextlib import ExitStack

import numpy as np

import concourse.bass as bass
import concourse.tile as tile
from concourse import bacc, bass_utils, mybir
from concourse.masks import make_identity

F32 = mybir.dt.float32
BF16 = mybir.dt.bfloat16
FP8 = mybir.dt.float8e4
AF = mybir.ActivationFunctionType
ALU = mybir.AluOpType
PM = mybir.MatmulPerfMode

B, S, D = 4, 2048, 1024
H, DH, MLP = 16, 64, 4096
P = 128
KD = D // P
FT = MLP // P
NQ = S // 2
ST = S // P
QTT = NQ // P
QSL = 512
NQS = NQ // QSL      # 2 slices
NPAIR = 8
EPS = 1e-6
DEBUG = bool(int(os.environ.get("KERNEL_DEBUG", "0")))

_CACHE = {}


def _build(generic=False):
    nc = bacc.Bacc(None, target_bir_lowering=False, debug=False, num_devices=8)

    xkv = nc.dram_tensor("xkv", [S, D], F32, kind="ExternalInput").ap()
    wq = nc.dram_tensor("wq", [P, NPAIR, KD, P], FP8, kind="ExternalInput").ap()
    wk = nc.dram_tensor("wk", [P, NPAIR, KD, P], FP8, kind="ExternalInput").ap()
    wv = nc.dram_tensor("wv", [P, NPAIR // 2, KD, 2 * P], FP8, kind="ExternalInput").ap()
    wo = nc.dram_tensor("wo", [P, KD, KD, P], FP8, kind="ExternalInput").ap()
    w1 = nc.dram_tensor("w1", [P, FT, 2, KD, P], FP8, kind="ExternalInput").ap()
    w2 = nc.dram_tensor("w2", [P, KD, FT, P], BF16, kind="ExternalInput").ap()
    b1v = nc.dram_tensor("b1v", [MLP], F32, kind="ExternalInput").ap()
    b2v = nc.dram_tensor("b2v", [D], F32, kind="ExternalInput").ap()
    out = nc.dram_tensor("out", [NQ, D], F32, kind="ExternalOutput").ap()
    if generic:
        bqv = nc.dram_tensor("bqv", [D], F32, kind="ExternalInput").ap()
        bkv = nc.dram_tensor("bkv", [D], F32, kind="ExternalInput").ap()
        bvv = nc.dram_tensor("bvv", [D], F32, kind="ExternalInput").ap()
        bov = nc.dram_tensor("bov", [D], F32, kind="ExternalInput").ap()
        g1v = nc.dram_tensor("g1v", [D], F32, kind="ExternalInput").ap()
        be1v = nc.dram_tensor("be1v", [D], F32, kind="ExternalInput").ap()
        g2v = nc.dram_tensor("g2v", [D], F32, kind="ExternalInput").ap()
        be2v = nc.dram_tensor("be2v", [D], F32, kind="ExternalInput").ap()

    dbg = {}
    if DEBUG:
        dbg["xnT"] = nc.dram_tensor("d_xnT", [P, KD, S], FP8, kind="ExternalOutput").ap()
        dbg["v8"] = nc.dram_tensor("d_v8", [P, NPAIR, ST // 2, 2, 2, 64], FP8, kind="ExternalOutput").ap()
        dbg["q0"] = nc.dram_tensor("d_q0", [P, 2, 2, QSL], FP8, kind="ExternalOutput").ap()
        dbg["e0"] = nc.dram_tensor("d_e0", [P, ST, 2, QSL], FP8, kind="ExternalOutput").ap()
        dbg["rt"] = nc.dram_tensor("d_rt", [P, NPAIR, NQ], FP8, kind="ExternalOutput").ap()
        dbg["x2"] = nc.dram_tensor("d_x2", [P, QTT, D], BF16, kind="ExternalOutput").ap()
        dbg["xn2T"] = nc.dram_tensor("d_xn2T", [P, KD, NQ], BF16, kind="ExternalOutput").ap()
        dbg["h1"] = nc.dram_tensor("d_h1", [P, FT, QSL], BF16, kind="ExternalOutput").ap()

    def bcast_ap(vec, n):
        return bass.AP(tensor=vec.tensor, offset=vec.offset,
                       ap=[[0, n]] + list(vec.ap))

    with tile.TileContext(nc) as tc:
        es = ExitStack()
        params = es.enter_context(tc.tile_pool(name="params", bufs=1))

        ident_f = params.tile([P, P], F32)
        make_identity(nc, ident_f)
        ident_b = params.tile([P, P], BF16)
        nc.vector.tensor_copy(ident_b[:], ident_f[:])

        def pvec(v, n, nm):
            t = params.tile([P, n], F32, name=nm)
            nc.sync.dma_start(t[:], v.rearrange("(o p) -> p o", p=P))
            return t

        b1_t = pvec(b1v, FT, "b1_t")
        b2_t = pvec(b2v, KD, "b2_t")
        eps_t = params.tile([P, 1], F32)
        nc.vector.memset(eps_t[:], EPS)
        expb_t = params.tile([P, 1], F32)
        nc.vector.memset(expb_t[:], -3.0)
        rc32_t = params.tile([P, 1], F32)
        nc.vector.memset(rc32_t[:], 1.0 / 32.0)
        if generic:
            bq_t = pvec(bqv, KD, "bq_t")
            bk_t = pvec(bkv, KD, "bk_t")
            bo_t = pvec(bov, KD, "bo_t")
            rc1024_t = params.tile([P, 1], F32, name="rc1024")
            nc.vector.memset(rc1024_t[:], 1.0 / 1024.0)
            bv_rep = params.tile([P, D], F32)
            nc.gpsimd.dma_start(bv_rep[:], bcast_ap(bvv, P))
            g1_rep = params.tile([P, D], F32)
            nc.gpsimd.dma_start(g1_rep[:], bcast_ap(g1v, P))
            be1_rep = params.tile([P, D], F32)
            nc.gpsimd.dma_start(be1_rep[:], bcast_ap(be1v, P))
            g2_rep = params.tile([P, D], F32)
            nc.gpsimd.dma_start(g2_rep[:], bcast_ap(g2v, P))
            be2_rep = params.tile([P, D], F32)
            nc.gpsimd.dma_start(be2_rep[:], bcast_ap(be2v, P))

        wqk_es = ExitStack()
        wqkp = wqk_es.enter_context(tc.tile_pool(name="wqk", bufs=1))
        wq_s = wqkp.tile([P, NPAIR, KD, P], FP8)
        wk_s = wqkp.tile([P, NPAIR, KD, P], FP8)
        wv_s = wqkp.tile([P, NPAIR // 2, KD, 2 * P], FP8)
        wo_s = wqkp.tile([P, KD, KD, P], FP8)
        nc.gpsimd.dma_start(wq_s[:], wq)
        nc.gpsimd.dma_start(wk_s[:], wk)
        nc.gpsimd.dma_start(wv_s[:], wv)
        nc.gpsimd.dma_start(wo_s[:], wo)

        xn_es = ExitStack()
        xnp = xn_es.enter_context(tc.tile_pool(name="xn", bufs=1))
        xnT8 = xnp.tile([P, KD, S], FP8)

        att_es = ExitStack()
        attp = att_es.enter_context(tc.tile_pool(name="attn", bufs=1))
        # split in halves: dual-fp8 ldweights can't address >16KB/partition strides
        KT8h = [attp.tile([P, NPAIR // 2, S + P], FP8, name=f"KT8h{i}")
                for i in range(2)]
        V8h = [attp.tile([P, NPAIR // 2, ST // 2, 2, 2, 64], FP8, name=f"V8h{i}")
               for i in range(2)]
        ones8 = attp.tile([P, 2, 64], FP8)   # lhsT for the denominator chain
        qpadA = attp.tile([P, 2, QSL], FP8)
        qpadB = attp.tile([P, 2, QSL], FP8)
        e8a = attp.tile([P, ST, 2, QSL], FP8)
        RT8 = [attp.tile([P, NPAIR, QSL], FP8, name=f"RT8_{s}") for s in range(NQS)]

        x2_es = ExitStack()
        x2p = x2_es.enter_context(tc.tile_pool(name="x2", bufs=1))
        xn2_es = ExitStack()
        xn2p = xn2_es.enter_context(tc.tile_pool(name="xn2", bufs=1))
        xn2T = [xn2p.tile([P, KD, QSL], FP8, name=f"xn2T{s}") for s in range(NQS)]

        with tc.tile_pool(name="zinit", bufs=1) as zp:
            zf = zp.tile([P, S], F32)
            nc.vector.memset(zf[:], 0.0)
            for i in range(2):
                nc.vector.tensor_copy(KT8h[i][:, :, S:S + P],
                                      zf[:, 0:(NPAIR // 2) * P])
            nc.vector.tensor_copy(qpadA[:, 1, :], zf[:, 0:QSL])
            nc.vector.tensor_copy(qpadB[:, 1, :], zf[:, 0:QSL])
            nc.vector.tensor_copy(qpadA[64:128, 0, :], zf[64:128, 0:QSL])
            nc.vector.tensor_copy(qpadB[0:64, 0, :], zf[0:64, 0:QSL])
            onef = zp.tile([P, 2 * 64], F32)
            nc.vector.memset(onef[:], 1.0)
            nc.vector.tensor_copy(ones8[:], onef[:])

        def vproj_t(nc_ps_pool, g, t):
            ps = nc_ps_pool.tile([P, 2 * P], F32, tag="vp")
            for j in range(KD // 2):
                nc.tensor.matmul(
                    ps[:], xnT8[:, 2 * j:2 * j + 2, t * P:(t + 1) * P],
                    wv_s[:, g, 2 * j:2 * j + 2, :],
                    start=(j == 0), stop=(j == KD // 2 - 1),
                    perf_mode=PM.DoubleRow)
            dst = V8h[g // 2][:, (2 * g) % 4:(2 * g) % 4 + 2, t // 2, t % 2, :, :]
            src_ = ps[:].rearrange("p (a h m) -> p a h m", a=2, h=2)
            if generic:
                nc.vector.tensor_tensor(
                    dst, src_,
                    bv_rep[:].rearrange("p (a h m) -> p a h m", a=KD, h=2)
                    [:, 2 * g:2 * g + 2], ALU.add)
            else:
                nc.vector.tensor_copy(dst, src_)

        # ---- LN1 over all 16 kv tiles -> xnT8 ----
        with tc.tile_pool(name="p1x", bufs=3) as p1x, \
             tc.tile_pool(name="p1s", bufs=4) as p1s, \
             tc.tile_pool(name="p1ps", bufs=4, space="PSUM") as ps1:
            for t in range(ST):
                x_t = p1x.tile([P, D], F32, tag="x_t")
                (nc.sync if t % 2 == 0 else nc.scalar).dma_start(
                    x_t[:], xkv[t * P:(t + 1) * P, :])
                stats = p1s.tile([P, 2, 6], F32, tag="stats")
                xv = x_t[:].rearrange("p (s f) -> p s f", s=2)
                for sh in range(2):
                    nc.vector.bn_stats(stats[:, sh, :], xv[:, sh, :])
                mv = p1s.tile([P, 2], F32, tag="mv")
                nc.vector.bn_aggr(mv[:], stats[:])
                std = p1s.tile([P, 1], F32, tag="std")
                nc.scalar.activation(std[:], mv[:, 1:2], AF.Sqrt, bias=eps_t[:])
                nc.vector.reciprocal(std[:], std[:])
                xn_t = p1x.tile([P, D], BF16, tag="xn_t")
                if generic:
                    xnf_t = p1x.tile([P, D], F32, tag="xnf_t")
                    nc.vector.tensor_scalar(
                        xnf_t[:], x_t[:], scalar1=mv[:, 0:1], scalar2=std[:],
                        op0=ALU.subtract, op1=ALU.mult)
                    nc.gpsimd.tensor_tensor(xnf_t[:], xnf_t[:], g1_rep[:], ALU.mult)
                    nc.gpsimd.tensor_tensor(xn_t[:], xnf_t[:], be1_rep[:], ALU.add)
                else:
                    nc.gpsimd.tensor_scalar(
                        xn_t[:], x_t[:], scalar1=mv[:, 0:1], scalar2=std[:],
                        op0=ALU.subtract, op1=ALU.mult)
                for j2 in range(KD // 2):
                    pst = ps1.tile([P, P], F32, tag="tp")
                    for hh in range(2):
                        nc.tensor.transpose(
                            pst[:].bitcast(BF16)[:, hh * P:(hh + 1) * P],
                            xn_t[:, (2 * j2 + hh) * P:(2 * j2 + hh + 1) * P],
                            ident_b[:])
                    nc.vector.tensor_copy(
                        xnT8[:, 2 * j2:2 * j2 + 2, t * P:(t + 1) * P],
                        pst[:].bitcast(BF16).rearrange("p (a m) -> p a m", a=2))

        if DEBUG:
            nc.sync.dma_start(dbg["xnT"], xnT8[:])

        with tc.tile_pool(name="scps", bufs=2, space="PSUM") as scp, \
             tc.tile_pool(name="avps", bufs=1, space="PSUM") as avp, \
             tc.tile_pool(name="pps", bufs=2, space="PSUM") as ppp, \
             tc.tile_pool(name="nrm", bufs=1) as nrmp, \
             tc.tile_pool(name="attT", bufs=1) as attTp, \
             tc.tile_pool(name="h1pool", bufs=1) as h1p, \
             tc.tile_pool(name="wst1", bufs=2) as wst1, \
             tc.tile_pool(name="wst2", bufs=2) as wst2, \
             tc.tile_pool(name="p4x", bufs=2) as p4x, \
             tc.tile_pool(name="p4y", bufs=1) as p4y, \
             tc.tile_pool(name="p4s", bufs=4) as p4s:

            def vproj(g):
                # V for pair group g (pairs 2g, 2g+1), all 16 kv tiles
                for t in range(ST):
                    ps = ppp.tile([P, QSL], F32, tag="pp")
                    for j in range(KD // 2):
                        nc.tensor.matmul(
                            ps[:, 0:2 * P],
                            xnT8[:, 2 * j:2 * j + 2, t * P:(t + 1) * P],
                            wv_s[:, g, 2 * j:2 * j + 2, :],
                            start=(j == 0), stop=(j == KD // 2 - 1),
                            perf_mode=PM.DoubleRow)
                    dst = V8h[g // 2][:, (2 * g) % 4:(2 * g) % 4 + 2,
                                      t // 2, t % 2, :, :]
                    src = ps[:, 0:2 * P].rearrange("p (a h m) -> p a h m", a=2, h=2)
                    if generic:
                        nc.vector.tensor_tensor(
                            dst, src,
                            bv_rep[:].rearrange("p (a h m) -> p a h m", a=KD, h=2)
                            [:, 2 * g:2 * g + 2], ALU.add)  # noqa
                    else:
                        nc.vector.tensor_copy(dst, src)

            def kproj_all(skip_pair0=True):
                for pr in range(NPAIR):
                    if skip_pair0 and pr == 0:
                        continue
                    for ksl in range(S // QSL):
                        ps = ppp.tile([P, QSL], F32, tag="pp")
                        for j in range(KD // 2):
                            nc.tensor.matmul(
                                ps[:], wk_s[:, pr, 2 * j:2 * j + 2, :],
                                xnT8[:, 2 * j:2 * j + 2, ksl * QSL:(ksl + 1) * QSL],
                                start=(j == 0), stop=(j == KD // 2 - 1),
                                perf_mode=PM.DoubleRow)
                        if generic:
                            nc.vector.tensor_scalar_add(
                                KT8h[pr // 4][:, pr % 4, ksl * QSL:(ksl + 1) * QSL],
                                ps[:], bk_t[:, pr:pr + 1])
                        else:
                            nc.vector.tensor_copy(
                                KT8h[pr // 4][:, pr % 4, ksl * QSL:(ksl + 1) * QSL],
                                ps[:])
                        yield

            def attention_slice(s, gen):
                q0 = s * QSL

                def adv(n=1):
                    if gen is not None:
                        for _ in range(n):
                            next(gen, None)

                for pr in range(NPAIR):
                    if s == 0 and pr % 2 == 0:
                        vproj(pr // 2)
                    # Q projection
                    ps = ppp.tile([P, QSL], F32, tag="pp")
                    for j in range(KD // 2):
                        nc.tensor.matmul(
                            ps[:], wq_s[:, pr, 2 * j:2 * j + 2, :],
                            xnT8[:, 2 * j:2 * j + 2, q0:q0 + QSL],
                            start=(j == 0), stop=(j == KD // 2 - 1),
                            perf_mode=PM.DoubleRow)
                    if generic:
                        nc.vector.tensor_scalar_add(
                            qpadA[0:64, 0, :], ps[0:64, :], bq_t[0:64, pr:pr + 1])
                        nc.vector.tensor_scalar_add(
                            qpadB[64:128, 0, :], ps[64:128, :],
                            bq_t[64:128, pr:pr + 1])
                    else:
                        nc.vector.tensor_copy(qpadA[0:64, 0, :], ps[0:64, :])
                        nc.vector.tensor_copy(qpadB[64:128, 0, :], ps[64:128, :])
                    adv(1)
                    if s == 0 and pr == 0:
                        # K for pair 0 inline; pairs 1..7 stream via the gen
                        for ksl in range(S // QSL):
                            ps = ppp.tile([P, QSL], F32, tag="pp")
                            for j in range(KD // 2):
                                nc.tensor.matmul(
                                    ps[:], wk_s[:, 0, 2 * j:2 * j + 2, :],
                                    xnT8[:, 2 * j:2 * j + 2,
                                         ksl * QSL:(ksl + 1) * QSL],
                                    start=(j == 0), stop=(j == KD // 2 - 1),
                                    perf_mode=PM.DoubleRow)
                            if generic:
                                nc.vector.tensor_scalar_add(
                                    KT8h[0][:, 0, ksl * QSL:(ksl + 1) * QSL], ps[:],
                                    bk_t[:, 0:1])
                            else:
                                nc.vector.tensor_copy(
                                    KT8h[0][:, 0, ksl * QSL:(ksl + 1) * QSL], ps[:])
                    if DEBUG and pr == 0 and s == 0:
                        nc.sync.dma_start(dbg["q0"][:, 0], qpadA[:])
                        nc.sync.dma_start(dbg["q0"][:, 1], qpadB[:])
                    # scores + exp per 2-kt group, split in q-halves so the
                    # two sc buffers double-buffer against the exp reads
                    QH = QSL // 2
                    for qh in range(2):
                        for c in range(ST // 2):
                            sc = scp.tile([P, 2, 2, QH], F32, tag="sc")
                            for jj in range(2):
                                kt = 2 * c + jj
                                ktv = KT8h[pr // 4][:, pr % 4,
                                                   kt * P:kt * P + 2 * P].rearrange(
                                    "p (a b) -> p a b", a=2)
                                nc.tensor.matmul(
                                    sc[:, jj, 0, :], ktv,
                                    qpadA[:, :, qh * QH:(qh + 1) * QH],
                                    start=True, stop=True, perf_mode=PM.DoubleRow)
                                nc.tensor.matmul(
                                    sc[:, jj, 1, :], ktv,
                                    qpadB[:, :, qh * QH:(qh + 1) * QH],
                                    start=True, stop=True, perf_mode=PM.DoubleRow)
                            nc.scalar.activation(
                                e8a[:, 2 * c:2 * c + 2, :, qh * QH:(qh + 1) * QH],
                                sc[:],
                                AF.Exp, bias=expb_t[:], scale=1.0 / 8192.0)
                        adv(2)
                    if DEBUG and pr == 0 and s == 0:
                        nc.sync.dma_start(dbg["e0"], e8a[:])
                    # AV (feature-major out) + broadcast-normalize
                    for hh in range(2):
                        av = avp.tile([P, QSL], F32, tag="av")
                        dn = avp.tile([P, QSL], F32, tag="dn")
                        for c in range(ST // 2):
                            st_, sp_ = (c == 0), (c == ST // 2 - 1)
                            nc.tensor.matmul(
                                av[0:64, :], V8h[pr // 4][:, pr % 4, c, :, hh, :],
                                e8a[:, 2 * c:2 * c + 2, hh, :],
                                start=st_, stop=sp_,
                                perf_mode=PM.DoubleRow, skip_group_check=True)
                            nc.tensor.matmul(
                                dn[0:64, :], ones8[:],
                                e8a[:, 2 * c:2 * c + 2, hh, :],
                                start=st_, stop=sp_,
                                perf_mode=PM.DoubleRow, skip_group_check=True)
                        den0 = nrmp.tile([1, QSL], F32, tag="den0")
                        nc.vector.reciprocal(den0[:], dn[0:1, :])
                        rb = nrmp.tile([64, QSL], F32, tag="rb")
                        nc.gpsimd.partition_broadcast(rb[:], den0[:])
                        nc.vector.tensor_tensor(
                            RT8[s][64 * hh:64 * hh + 64, pr, :],
                            av[0:64, :], rb[:], ALU.mult)
                        adv(1)
                if DEBUG and s == NQS - 1:
                    for si in range(NQS):
                        nc.sync.dma_start(
                            dbg["rt"].rearrange("p k (a q) -> p k a q", a=NQS)[:, :, si, :],
                            RT8[si][:])

            def mlp_slice(s):
                q0 = s * QSL
                x2_s = x2p.tile([P, QSL // P, D], BF16, tag="x2s")
                attnT = attTp.tile([P, KD, QSL], BF16, tag="attnT")
                for mt in range(KD):
                    ps = ppp.tile([P, QSL], F32, tag="pp")
                    for j in range(KD // 2):
                        nc.tensor.matmul(
                            ps[:], wo_s[:, mt, 2 * j:2 * j + 2, :],
                            RT8[s][:, 2 * j:2 * j + 2, :],
                            start=(j == 0), stop=(j == KD // 2 - 1),
                            perf_mode=PM.DoubleRow)
                    if generic:
                        nc.vector.tensor_scalar(
                            attnT[:, mt, :], ps[:],
                            scalar1=rc1024_t[:], scalar2=bo_t[:, mt:mt + 1],
                            op0=ALU.mult, op1=ALU.add)
                    else:
                        nc.vector.tensor_single_scalar(
                            attnT[:, mt, :], ps[:], 1.0 / 1024.0, ALU.mult)
                    yield
                # x2 + LN2 stats (sqrt batched per slice)
                mvs = p4s.tile([P, QSL // P, 2], F32, tag="mvs")
                for tloc in range(QSL // P):
                    tt = s * (QSL // P) + tloc
                    x_t = p4x.tile([P, D], F32, tag="xr_t")
                    nc.sync.dma_start(x_t[:], xkv[tt * P:(tt + 1) * P, :])
                    pst = ppp.tile([P, QSL], F32, tag="pp")
                    for mt in range(KD):
                        nc.tensor.transpose(
                            pst[:].bitcast(BF16)[:, mt * P:(mt + 1) * P],
                            attnT[:, mt, tloc * P:(tloc + 1) * P], ident_b[:])
                    nc.vector.tensor_tensor(
                        x2_s[:, tloc, :], pst[:].bitcast(BF16), x_t[:], ALU.add)
                    stats = p4s.tile([P, 2, 6], F32, tag="stats2")
                    xv = x2_s[:, tloc, :].rearrange("p (a f) -> p a f", a=2)
                    for sh in range(2):
                        nc.vector.bn_stats(stats[:, sh, :], xv[:, sh, :])
                    nc.vector.bn_aggr(mvs[:, tloc, :], stats[:])
                    yield
                stds = p4s.tile([P, QSL // P], F32, tag="stds")
                nc.scalar.activation(
                    stds[:], mvs[:].rearrange("p a b -> p (a b)")[:, 1::2],
                    AF.Sqrt, bias=eps_t[:])
                nc.vector.reciprocal(stds[:], stds[:])
                for tloc in range(QSL // P):
                    tt = s * (QSL // P) + tloc
                    xn2_t = p4y.tile([P, D], BF16, tag="xn2_t")
                    if generic:
                        xn2f = p4y.tile([P, D], F32, tag="xn2f")
                        nc.vector.tensor_scalar(
                            xn2f[:], x2_s[:, tloc, :], scalar1=mvs[:, tloc, 0:1],
                            scalar2=stds[:, tloc:tloc + 1],
                            op0=ALU.subtract, op1=ALU.mult)
                        nc.gpsimd.tensor_tensor(xn2f[:], xn2f[:], g2_rep[:], ALU.mult)
                        nc.gpsimd.tensor_tensor(xn2_t[:], xn2f[:], be2_rep[:], ALU.add)
                    else:
                        nc.vector.tensor_scalar(
                            xn2_t[:], x2_s[:, tloc, :], scalar1=mvs[:, tloc, 0:1],
                            scalar2=stds[:, tloc:tloc + 1],
                            op0=ALU.subtract, op1=ALU.mult)
                    for j2 in range(KD // 2):
                        pst2 = ppp.tile([P, QSL], F32, tag="pp")
                        for hh in range(2):
                            nc.tensor.transpose(
                                pst2[:].bitcast(BF16)[:, hh * P:(hh + 1) * P],
                                xn2_t[:, (2 * j2 + hh) * P:(2 * j2 + hh + 1) * P],
                                ident_b[:])
                        nc.vector.tensor_copy(
                            xn2T[s][:, 2 * j2:2 * j2 + 2, tloc * P:(tloc + 1) * P],
                            pst2[:].bitcast(BF16)[:, 0:2 * P].rearrange(
                                "p (a m) -> p a m", a=2))
                    yield
                if DEBUG:
                    nc.sync.dma_start(
                        dbg["xn2T"].rearrange("p k (a q) -> p k a q", a=NQS)[:, :, s, :],
                        xn2T[s][:])
                    if s == 0:
                        nc.sync.dma_start(dbg["x2"][:, 0:QSL // P, :], x2_s[:])
                # h1: bf16 matmuls, bias in the psum copy, gelu deferred+batched
                h1T = h1p.tile([P, FT, QSL], BF16, tag="h1T")
                for fg in range(FT // 2):
                    w1c = wst1.tile([P, 2, 2, KD, P], FP8, tag="w1c")
                    nc.sync.dma_start(w1c[:], w1[:, 2 * fg:2 * fg + 2])
                    for f2 in range(2):
                        ft = 2 * fg + f2
                        ps = ppp.tile([P, QSL], F32, tag="pp")
                        for half in range(2):
                            for j in range(KD // 2):
                                nc.tensor.matmul(
                                    ps[:], w1c[:, f2, half, 2 * j:2 * j + 2, :],
                                    xn2T[s][:, 2 * j:2 * j + 2, :],
                                    start=(half == 0 and j == 0),
                                    stop=(half == 1 and j == KD // 2 - 1),
                                    perf_mode=PM.DoubleRow)
                        nc.vector.tensor_scalar(
                            h1T[:, ft, :], ps[:], scalar1=rc32_t[:],
                            scalar2=b1_t[:, ft:ft + 1], op0=ALU.mult, op1=ALU.add)
                        yield
                for gg in range(4):
                    nc.scalar.activation(
                        h1T[:, 8 * gg:8 * gg + 8, :],
                        h1T[:, 8 * gg:8 * gg + 8, :], AF.Gelu)
                yield
                if DEBUG and s == 0:
                    nc.sync.dma_start(dbg["h1"], h1T[:])
                # h2 + transpose + final residual + store
                outT = attTp.tile([P, KD, QSL], BF16, tag="attnT")
                for mt in range(KD):
                    ps = ppp.tile([P, QSL], F32, tag="pp")
                    for fh in range(2):
                        w2c = wst2.tile([P, FT // 2, P], BF16, tag="w2c")
                        nc.sync.dma_start(
                            w2c[:], w2[:, mt, fh * (FT // 2):(fh + 1) * (FT // 2)])
                        for fi in range(FT // 2):
                            ft = fh * (FT // 2) + fi
                            nc.tensor.matmul(
                                ps[:], w2c[:, fi, :], h1T[:, ft, :],
                                start=(ft == 0), stop=(ft == FT - 1))
                    nc.vector.tensor_scalar_add(
                        outT[:, mt, :], ps[:], b2_t[:, mt:mt + 1])
                    yield
                for tloc in range(QSL // P):
                    tt = s * (QSL // P) + tloc
                    pst = ppp.tile([P, QSL], F32, tag="pp")
                    for mt in range(KD):
                        nc.tensor.transpose(
                            pst[:].bitcast(BF16)[:, mt * P:(mt + 1) * P],
                            outT[:, mt, tloc * P:(tloc + 1) * P], ident_b[:])
                    ob = p4y.tile([P, D], F32, tag="ob")
                    nc.vector.tensor_tensor(
                        ob[:], pst[:].bitcast(BF16), x2_s[:, tloc, :], ALU.add)
                    nc.sync.dma_start(out[tt * P:(tt + 1) * P, :], ob[:])
                    yield

            gen = kproj_all()
            for s in range(NQS):
                attention_slice(s, gen)
                if gen is not None:
                    for _ in gen:
                        pass
                gen = mlp_slice(s)
            for _ in gen:
                pass

        xn2_es.close()
        x2_es.close()
        att_es.close()
        xn_es.close()
        wqk_es.close()
        es.close()

    nc.compile()
    return nc


def kernel(**inputs):
    import ml_dtypes
    E4 = ml_dtypes.float8_e4m3

    inputs = {k: np.ascontiguousarray(np.asarray(v), dtype=np.float32)
              for k, v in inputs.items()}
    generic = not (
        np.all(inputs["ln1_g"] == 1.0) and np.all(inputs["ln1_b"] == 0.0)
        and np.all(inputs["ln2_g"] == 1.0) and np.all(inputs["ln2_b"] == 0.0)
        and np.all(inputs["bq"] == 0.0) and np.all(inputs["bk"] == 0.0)
        and np.all(inputs["bv"] == 0.0) and np.all(inputs["bo"] == 0.0))
    key = ("nc", generic)
    if key not in _CACHE:
        _CACHE[key] = _build(generic=generic)
    nc = _CACHE[key]

    x = inputs["x"]

    def tile8(W, npart):
        Din, Dout = W.shape
        t = (W * 32.0).reshape(Din // P, P, npart, Dout // npart).transpose(1, 2, 0, 3)
        return np.ascontiguousarray(t.astype(E4))

    def tile8s(W, nt):
        Din, Dout = W.shape
        t = (W * 32.0).reshape(Din // P, P, nt, Dout // nt).transpose(1, 2, 0, 3)
        hi = t.astype(E4)
        lo = (t - hi.astype(np.float32)).astype(E4)
        return np.ascontiguousarray(
            np.stack([hi, lo], axis=2))  # [P, nt, 2, KD, m]

    def tileb(W, nt):
        Din, Dout = W.shape
        t = W.reshape(Din // P, P, nt, Dout // nt).transpose(1, 2, 0, 3)
        return np.ascontiguousarray(t.astype(ml_dtypes.bfloat16))

    shared = {
        "wq": tile8(inputs["Wq"], NPAIR), "wk": tile8(inputs["Wk"], NPAIR),
        "wv": tile8(inputs["Wv"], NPAIR // 2), "wo": tile8(inputs["Wo"], KD),
        "w1": tile8s(inputs["W1"], FT), "w2": tileb(inputs["W2"], KD),
        "b1v": inputs["b1"], "b2v": inputs["b2"],
    }
    if generic:
        shared.update({
            "bqv": inputs["bq"] * 32.0, "bkv": inputs["bk"] * 32.0,
            "bvv": inputs["bv"] * 32.0, "bov": inputs["bo"],
            "g1v": inputs["ln1_g"], "be1v": inputs["ln1_b"],
            "g2v": inputs["ln2_g"], "be2v": inputs["ln2_b"],
        })

    in_maps = []
    for c in range(8):
        b, half = c // 2, c % 2
        m = dict(shared)
        m["xkv"] = np.ascontiguousarray(
            np.concatenate([x[b, half * NQ:(half + 1) * NQ, :],
                            x[b, (1 - half) * NQ:(2 - half) * NQ, :]], axis=0))
        in_maps.append(m)

    res = bass_utils.run_bass_kernel_spmd(nc, in_maps, core_ids=list(range(8)))
    _CACHE["last_results"] = res
    _CACHE["nc"] = nc
    _CACHE["last_in_maps"] = in_maps

    outa = np.empty((B, S, D), dtype=np.float32)
    for c in range(8):
        b, half = c // 2, c % 2
        outa[b, half * NQ:(half + 1) * NQ, :] = res.results[c]["out"]
    return outa


# revision 8
# speedup vs baseline: 1.0135x; 1.0135x over previous
"""Trainium2 Bass kernel v4 for the dense transformer encoder block
(B=4, S=2048, D=1024, H=16, MLP=4096). 8 cores = 4 batch x 2 query-halves,
kv host-reordered so each core's 1024 queries come first.

630,966 ns (TimelineSim) vs the 861,764 ns f32r baseline; rel err 1.48e-2.

- fp8(e4m3) DoubleRow matmuls (0.5 cy/row, 2x128-deep per instr) for the
  attention path (Q/K/V/O projections, scores, AV). x32 weight prescale
  (exact power of 2) keeps e4m3 out of subnormals; the 1/32 factors fold
  into the exp scale (1/8192) and O-proj copy (1/1024). Scores pad dh=64
  via zeroed qpad rows/k-tiles (cost is output-column driven, padding is
  free); the DR tile1 of the K operand points at the next k-tile's data
  (legal because the matching rhs tile is zero).
- AV feature-major: out[64, q] plus a parallel all-ones-lhsT chain giving
  the softmax denominator (dual-fp8 ldweights only allows 64/128-wide
  tiles, and DR psum targets must start at partition 0). Normalize = DVE
  reciprocal + Pool partition_broadcast + one DVE multiply out of PSUM;
  attention output lands feature-major so O-proj needs no transposes.
- exp on ACT: scale 1/8192, bias -3 (e4m3 overflow headroom). The scores
  psum is two half-q-width [128, 2kt, 2h, 256] tiles used alternately so
  score matmuls double-buffer against the exp reads.
- h1 = fp8 DoubleRow with error-split weights (W1 = fp8-hi + fp8-lo, both
  chained into one psum) over fp8 xn2T; h2 stays bf16 (the gelu output
  cannot be weight-split and plain fp8 exceeds 2e-2). W1/W2 streamed in
  double-buffered chunks (64KB/partition each - cannot be resident). Gelu
  deferred and batched in-place (bias added in the psum->bf16 copy) and
  LN2's sqrt batched per slice, so the ACT table rarely reloads.
- 2-slice software pipeline over the 1024 queries: slice s's ACT-bound
  softmax overlaps slice s-1's PE-bound MLP via a chunked generator
  (~1-3us chunks consumed at the exp stall points); K-projections are
  slice-invariant, hoisted, and streamed into slice 0's stalls.
"""

import os
import sys

sys.path.insert(0, "/opt/trn_rl_repo")

from contextlib import ExitStack

import numpy as np

import concourse.bass as bass
import concourse.tile as tile
from concourse import bacc, bass_utils, mybir
from concourse.masks import make_identity

F32 = mybir.dt.float32
BF16 = mybir.dt.bfloat16
FP8 = mybir.dt.float8e4
AF = mybir.ActivationFunctionType
ALU = mybir.AluOpType
PM = mybir.MatmulPerfMode

B, S, D = 4, 2048, 1024
H, DH, MLP = 16, 64, 4096
P = 128
KD = D // P
FT = MLP // P
NQ = S // 2
ST = S // P
QTT = NQ // P
QSL = 512
NQS = NQ // QSL      # 2 slices
NPAIR = 8
EPS = 1e-6
DEBUG = bool(int(os.environ.get("KERNEL_DEBUG", "0")))

_CACHE = {}


def _build(generic=False):
    nc = bacc.Bacc(None, target_bir_lowering=False, debug=False, num_devices=8)

    xkv = nc.dram_tensor("xkv", [S, D], F32, kind="ExternalInput").ap()
    wq = nc.dram_tensor("wq", [P, NPAIR, KD, P], FP8, kind="ExternalInput").ap()
    wk = nc.dram_tensor("wk", [P, NPAIR, KD, P], FP8, kind="ExternalInput").ap()
    wv = nc.dram_tensor("wv", [P, NPAIR // 2, KD, 2 * P], FP8, kind="ExternalInput").ap()
    wo = nc.dram_tensor("wo", [P, KD, KD, P], FP8, kind="ExternalInput").ap()
    w1 = nc.dram_tensor("w1", [P, FT, 2, KD, P], FP8, kind="ExternalInput").ap()
    w2 = nc.dram_tensor("w2", [P, KD, FT, P], BF16, kind="ExternalInput").ap()
    b1v = nc.dram_tensor("b1v", [MLP], F32, kind="ExternalInput").ap()
    b2v = nc.dram_tensor("b2v", [D], F32, kind="ExternalInput").ap()
    out = nc.dram_tensor("out", [NQ, D], F32, kind="ExternalOutput").ap()
    if generic:
        bqv = nc.dram_tensor("bqv", [D], F32, kind="ExternalInput").ap()
        bkv = nc.dram_tensor("bkv", [D], F32, kind="ExternalInput").ap()
        bvv = nc.dram_tensor("bvv", [D], F32, kind="ExternalInput").ap()
        bov = nc.dram_tensor("bov", [D], F32, kind="ExternalInput").ap()
        g1v = nc.dram_tensor("g1v", [D], F32, kind="ExternalInput").ap()
        be1v = nc.dram_tensor("be1v", [D], F32, kind="ExternalInput").ap()
        g2v = nc.dram_tensor("g2v", [D], F32, kind="ExternalInput").ap()
        be2v = nc.dram_tensor("be2v", [D], F32, kind="ExternalInput").ap()

    dbg = {}
    if DEBUG:
        dbg["xnT"] = nc.dram_tensor("d_xnT", [P, KD, S], FP8, kind="ExternalOutput").ap()
        dbg["v8"] = nc.dram_tensor("d_v8", [P, NPAIR, ST // 2, 2, 2, 64], FP8, kind="ExternalOutput").ap()
        dbg["q0"] = nc.dram_tensor("d_q0", [P, 2, 2, QSL], FP8, kind="ExternalOutput").ap()
        dbg["e0"] = nc.dram_tensor("d_e0", [P, ST, 2, QSL], FP8, kind="ExternalOutput").ap()
        dbg["rt"] = nc.dram_tensor("d_rt", [P, NPAIR, NQ], FP8, kind="ExternalOutput").ap()
        dbg["x2"] = nc.dram_tensor("d_x2", [P, QTT, D], BF16, kind="ExternalOutput").ap()
        dbg["xn2T"] = nc.dram_tensor("d_xn2T", [P, KD, NQ], BF16, kind="ExternalOutput").ap()
        dbg["h1"] = nc.dram_tensor("d_h1", [P, FT, QSL], BF16, kind="ExternalOutput").ap()

    def bcast_ap(vec, n):
        return bass.AP(tensor=vec.tensor, offset=vec.offset,
                       ap=[[0, n]] + list(vec.ap))

    with tile.TileContext(nc) as tc:
        es = ExitStack()
        params = es.enter_context(tc.tile_pool(name="params", bufs=1))

        ident_f = params.tile([P, P], F32)
        make_identity(nc, ident_f)
        ident_b = params.tile([P, P], BF16)
        nc.vector.tensor_copy(ident_b[:], ident_f[:])

        def pvec(v, n, nm):
            t = params.tile([P, n], F32, name=nm)
            nc.sync.dma_start(t[:], v.rearrange("(o p) -> p o", p=P))
            return t

        b1_t = pvec(b1v, FT, "b1_t")
        b2_t = pvec(b2v, KD, "b2_t")
        eps_t = params.tile([P, 1], F32)
        nc.vector.memset(eps_t[:], EPS)
        expb_t = params.tile([P, 1], F32)
        nc.vector.memset(expb_t[:], -3.0)
        rc32_t = params.tile([P, 1], F32)
        nc.vector.memset(rc32_t[:], 1.0 / 32.0)
        if generic:
            bq_t = pvec(bqv, KD, "bq_t")
            bk_t = pvec(bkv, KD, "bk_t")
            bo_t = pvec(bov, KD, "bo_t")
            rc1024_t = params.tile([P, 1], F32, name="rc1024")
            nc.vector.memset(rc1024_t[:], 1.0 / 1024.0)
            bv_rep = params.tile([P, D], F32)
            nc.gpsimd.dma_start(bv_rep[:], bcast_ap(bvv, P))
            g1_rep = params.tile([P, D], F32)
            nc.gpsimd.dma_start(g1_rep[:], bcast_ap(g1v, P))
            be1_rep = params.tile([P, D], F32)
            nc.gpsimd.dma_start(be1_rep[:], bcast_ap(be1v, P))
            g2_rep = params.tile([P, D], F32)
            nc.gpsimd.dma_start(g2_rep[:], bcast_ap(g2v, P))
            be2_rep = params.tile([P, D], F32)
            nc.gpsimd.dma_start(be2_rep[:], bcast_ap(be2v, P))

        wqk_es = ExitStack()
        wqkp = wqk_es.enter_context(tc.tile_pool(name="wqk", bufs=1))
        wq_s = wqkp.tile([P, NPAIR, KD, P], FP8)
        wk_s = wqkp.tile([P, NPAIR, KD, P], FP8)
        wv_s = wqkp.tile([P, NPAIR // 2, KD, 2 * P], FP8)
        wo_s = wqkp.tile([P, KD, KD, P], FP8)
        nc.gpsimd.dma_start(wq_s[:], wq)
        nc.gpsimd.dma_start(wk_s[:], wk)
        nc.gpsimd.dma_start(wv_s[:], wv)
        nc.gpsimd.dma_start(wo_s[:], wo)

        xn_es = ExitStack()
        xnp = xn_es.enter_context(tc.tile_pool(name="xn", bufs=1))
        xnT8 = xnp.tile([P, KD, S], FP8)

        att_es = ExitStack()
        attp = att_es.enter_context(tc.tile_pool(name="attn", bufs=1))
        # split in halves: dual-fp8 ldweights can't address >16KB/partition strides
        KT8h = [attp.tile([P, NPAIR // 2, S + P], FP8, name=f"KT8h{i}")
                for i in range(2)]
        V8h = [attp.tile([P, NPAIR // 2, ST // 2, 2, 2, 64], FP8, name=f"V8h{i}")
               for i in range(2)]
        ones8 = attp.tile([P, 2, 64], FP8)   # lhsT for the denominator chain
        qpadA = attp.tile([P, 2, QSL], FP8)
        qpadB = attp.tile([P, 2, QSL], FP8)
        e8a = attp.tile([P, ST, 2, QSL], FP8)
        RT8 = [attp.tile([P, NPAIR, QSL], FP8, name=f"RT8_{s}") for s in range(NQS)]

        x2_es = ExitStack()
        x2p = x2_es.enter_context(tc.tile_pool(name="x2", bufs=1))
        xn2_es = ExitStack()
        xn2p = xn2_es.enter_context(tc.tile_pool(name="xn2", bufs=1))
        xn2T = [xn2p.tile([P, KD, QSL], FP8, name=f"xn2T{s}") for s in range(NQS)]

        with tc.tile_pool(name="zinit", bufs=1) as zp:
            zf = zp.tile([P, S], F32)
            nc.vector.memset(zf[:], 0.0)
            for i in range(2):
                nc.vector.tensor_copy(KT8h[i][:, :, S:S + P],
                                      zf[:, 0:(NPAIR // 2) * P])
            nc.vector.tensor_copy(qpadA[:, 1, :], zf[:, 0:QSL])
            nc.vector.tensor_copy(qpadB[:, 1, :], zf[:, 0:QSL])
            nc.vector.tensor_copy(qpadA[64:128, 0, :], zf[64:128, 0:QSL])
            nc.vector.tensor_copy(qpadB[0:64, 0, :], zf[0:64, 0:QSL])
            onef = zp.tile([P, 2 * 64], F32)
            nc.vector.memset(onef[:], 1.0)
            nc.vector.tensor_copy(ones8[:], onef[:])

        def vproj_t(nc_ps_pool, g, t):
            ps = nc_ps_pool.tile([P, 2 * P], F32, tag="vp")
            for j in range(KD // 2):
                nc.tensor.matmul(
                    ps[:], xnT8[:, 2 * j:2 * j + 2, t * P:(t + 1) * P],
                    wv_s[:, g, 2 * j:2 * j + 2, :],
                    start=(j == 0), stop=(j == KD // 2 - 1),
                    perf_mode=PM.DoubleRow)
            dst = V8h[g // 2][:, (2 * g) % 4:(2 * g) % 4 + 2, t // 2, t % 2, :, :]
            src_ = ps[:].rearrange("p (a h m) -> p a h m", a=2, h=2)
            if generic:
                nc.vector.tensor_tensor(
                    dst, src_,
                    bv_rep[:].rearrange("p (a h m) -> p a h m", a=KD, h=2)
                    [:, 2 * g:2 * g + 2], ALU.add)
            else:
                nc.vector.tensor_copy(dst, src_)

        # ---- LN1 over all 16 kv tiles -> xnT8 ----
        with tc.tile_pool(name="p1x", bufs=3) as p1x, \
             tc.tile_pool(name="p1s", bufs=4) as p1s, \
             tc.tile_pool(name="p1ps", bufs=4, space="PSUM") as ps1:
            for t in range(ST):
                x_t = p1x.tile([P, D], F32, tag="x_t")
                (nc.sync if t % 2 == 0 else nc.scalar).dma_start(
                    x_t[:], xkv[t * P:(t + 1) * P, :])
                stats = p1s.tile([P, 2, 6], F32, tag="stats")
                xv = x_t[:].rearrange("p (s f) -> p s f", s=2)
                for sh in range(2):
                    nc.vector.bn_stats(stats[:, sh, :], xv[:, sh, :])
                mv = p1s.tile([P, 2], F32, tag="mv")
                nc.vector.bn_aggr(mv[:], stats[:])
                std = p1s.tile([P, 1], F32, tag="std")
                nc.scalar.activation(std[:], mv[:, 1:2], AF.Sqrt, bias=eps_t[:])
                nc.vector.reciprocal(std[:], std[:])
                xn_t = p1x.tile([P, D], BF16, tag="xn_t")
                if generic:
                    xnf_t = p1x.tile([P, D], F32, tag="xnf_t")
                    nc.vector.tensor_scalar(
                        xnf_t[:], x_t[:], scalar1=mv[:, 0:1], scalar2=std[:],
                        op0=ALU.subtract, op1=ALU.mult)
                    nc.gpsimd.tensor_tensor(xnf_t[:], xnf_t[:], g1_rep[:], ALU.mult)
                    nc.gpsimd.tensor_tensor(xn_t[:], xnf_t[:], be1_rep[:], ALU.add)
                else:
                    nc.gpsimd.tensor_scalar(
                        xn_t[:], x_t[:], scalar1=mv[:, 0:1], scalar2=std[:],
                        op0=ALU.subtract, op1=ALU.mult)
                for j2 in range(KD // 2):
                    pst = ps1.tile([P, P], F32, tag="tp")
                    for hh in range(2):
                        nc.tensor.transpose(
                            pst[:].bitcast(BF16)[:, hh * P:(hh + 1) * P],
                            xn_t[:, (2 * j2 + hh) * P:(2 * j2 + hh + 1) * P],
                            ident_b[:])
                    nc.vector.tensor_copy(
                        xnT8[:, 2 * j2:2 * j2 + 2, t * P:(t + 1) * P],
                        pst[:].bitcast(BF16).rearrange("p (a m) -> p a m", a=2))

        if DEBUG:
            nc.sync.dma_start(dbg["xnT"], xnT8[:])

        with tc.tile_pool(name="scps", bufs=2, space="PSUM") as scp, \
             tc.tile_pool(name="avps", bufs=1, space="PSUM") as avp, \
             tc.tile_pool(name="pps", bufs=2, space="PSUM") as ppp, \
             tc.tile_pool(name="nrm", bufs=1) as nrmp, \
             tc.tile_pool(name="attT", bufs=1) as attTp, \
             tc.tile_pool(name="h1pool", bufs=1) as h1p, \
             tc.tile_pool(name="wst1", bufs=2) as wst1, \
             tc.tile_pool(name="wst2", bufs=2) as wst2, \
             tc.tile_pool(name="p4x", bufs=2) as p4x, \
             tc.tile_pool(name="p4y", bufs=1) as p4y, \
             tc.tile_pool(name="p4s", bufs=4) as p4s:

            def vproj(g):
                # V for pair group g (pairs 2g, 2g+1), all 16 kv tiles
                for t in range(ST):
                    ps = ppp.tile([P, QSL], F32, tag="pp")
                    for j in range(KD // 2):
                        nc.tensor.matmul(
                            ps[:, 0:2 * P],
                            xnT8[:, 2 * j:2 * j + 2, t * P:(t + 1) * P],
                            wv_s[:, g, 2 * j:2 * j + 2, :],
                            start=(j == 0), stop=(j == KD // 2 - 1),
                            perf_mode=PM.DoubleRow)
                    dst = V8h[g // 2][:, (2 * g) % 4:(2 * g) % 4 + 2,
                                      t // 2, t % 2, :, :]
                    src = ps[:, 0:2 * P].rearrange("p (a h m) -> p a h m", a=2, h=2)
                    if generic:
                        nc.vector.tensor_tensor(
                            dst, src,
                            bv_rep[:].rearrange("p (a h m) -> p a h m", a=KD, h=2)
                            [:, 2 * g:2 * g + 2], ALU.add)  # noqa
                    else:
                        nc.vector.tensor_copy(dst, src)

            def kproj_all(skip_pair0=True):
                for pr in range(NPAIR):
                    if skip_pair0 and pr == 0:
                        continue
                    for ksl in range(S // QSL):
                        ps = ppp.tile([P, QSL], F32, tag="pp")
                        for j in range(KD // 2):
                            nc.tensor.matmul(
                                ps[:], wk_s[:, pr, 2 * j:2 * j + 2, :],
                                xnT8[:, 2 * j:2 * j + 2, ksl * QSL:(ksl + 1) * QSL],
                                start=(j == 0), stop=(j == KD // 2 - 1),
                                perf_mode=PM.DoubleRow)
                        if generic:
                            nc.vector.tensor_scalar_add(
                                KT8h[pr // 4][:, pr % 4, ksl * QSL:(ksl + 1) * QSL],
                                ps[:], bk_t[:, pr:pr + 1])
                        else:
                            nc.vector.tensor_copy(
                                KT8h[pr // 4][:, pr % 4, ksl * QSL:(ksl + 1) * QSL],
                                ps[:])
                        yield

            def attention_slice(s, gen):
                q0 = s * QSL

                def adv(n=1):
                    if gen is not None:
                        for _ in range(n):
                            next(gen, None)

                for pr in range(NPAIR):
                    if s == 0 and pr % 2 == 0:
                        vproj(pr // 2)
                    # Q projection
                    ps = ppp.tile([P, QSL], F32, tag="pp")
                    for j in range(KD // 2):
                        nc.tensor.matmul(
                            ps[:], wq_s[:, pr, 2 * j:2 * j + 2, :],
                            xnT8[:, 2 * j:2 * j + 2, q0:q0 + QSL],
                            start=(j == 0), stop=(j == KD // 2 - 1),
                            perf_mode=PM.DoubleRow)
                    if generic:
                        nc.vector.tensor_scalar_add(
                            qpadA[0:64, 0, :], ps[0:64, :], bq_t[0:64, pr:pr + 1])
                        nc.vector.tensor_scalar_add(
                            qpadB[64:128, 0, :], ps[64:128, :],
                            bq_t[64:128, pr:pr + 1])
                    else:
                        nc.vector.tensor_copy(qpadA[0:64, 0, :], ps[0:64, :])
                        nc.vector.tensor_copy(qpadB[64:128, 0, :], ps[64:128, :])
                    adv(1)
                    if s == 0 and pr == 0:
                        # K for pair 0 inline; pairs 1..7 stream via the gen
                        for ksl in range(S // QSL):
                            ps = ppp.tile([P, QSL], F32, tag="pp")
                            for j in range(KD // 2):
                                nc.tensor.matmul(
                                    ps[:], wk_s[:, 0, 2 * j:2 * j + 2, :],
                                    xnT8[:, 2 * j:2 * j + 2,
                                         ksl * QSL:(ksl + 1) * QSL],
                                    start=(j == 0), stop=(j == KD // 2 - 1),
                                    perf_mode=PM.DoubleRow)
                            if generic:
                                nc.vector.tensor_scalar_add(
                                    KT8h[0][:, 0, ksl * QSL:(ksl + 1) * QSL], ps[:],
                                    bk_t[:, 0:1])
                            else:
                                nc.vector.tensor_copy(
                                    KT8h[0][:, 0, ksl * QSL:(ksl + 1) * QSL], ps[:])
                    if DEBUG and pr == 0 and s == 0:
                        nc.sync.dma_start(dbg["q0"][:, 0], qpadA[:])
                        nc.sync.dma_start(dbg["q0"][:, 1], qpadB[:])
                    # scores + exp per 2-kt group, split in q-halves so the
                    # two sc buffers double-buffer against the exp reads
                    QH = QSL // 2
                    for qh in range(2):
                        for c in range(ST // 2):
                            sc = scp.tile([P, 2, 2, QH], F32, tag="sc")
                            for jj in range(2):
                                kt = 2 * c + jj
                                ktv = KT8h[pr // 4][:, pr % 4,
                                                   kt * P:kt * P + 2 * P].rearrange(
                                    "p (a b) -> p a b", a=2)
                                nc.tensor.matmul(
                                    sc[:, jj, 0, :], ktv,
                                    qpadA[:, :, qh * QH:(qh + 1) * QH],
                                    start=True, stop=True, perf_mode=PM.DoubleRow)
                                nc.tensor.matmul(
                                    sc[:, jj, 1, :], ktv,
                                    qpadB[:, :, qh * QH:(qh + 1) * QH],
                                    start=True, stop=True, perf_mode=PM.DoubleRow)
                            nc.scalar.activation(
                                e8a[:, 2 * c:2 * c + 2, :, qh * QH:(qh + 1) * QH],
                                sc[:],
                                AF.Exp, bias=expb_t[:], scale=1.0 / 8192.0)
                        adv(2)
                    if DEBUG and pr == 0 and s == 0:
                        nc.sync.dma_start(dbg["e0"], e8a[:])
                    # AV (feature-major out) + broadcast-normalize
                    for hh in range(2):
                        av = avp.tile([P, QSL], F32, tag="av")
                        dn = avp.tile([P, QSL], F32, tag="dn")
                        for c in range(ST // 2):
                            st_, sp_ = (c == 0), (c == ST // 2 - 1)
                            nc.tensor.matmul(
                                av[0:64, :], V8h[pr // 4][:, pr % 4, c, :, hh, :],
                                e8a[:, 2 * c:2 * c + 2, hh, :],
                                start=st_, stop=sp_,
                                perf_mode=PM.DoubleRow, skip_group_check=True)
                            nc.tensor.matmul(
                                dn[0:64, :], ones8[:],
                                e8a[:, 2 * c:2 * c + 2, hh, :],
                                start=st_, stop=sp_,
                                perf_mode=PM.DoubleRow, skip_group_check=True)
                        den0 = nrmp.tile([1, QSL], F32, tag="den0")
                        nc.vector.reciprocal(den0[:], dn[0:1, :])
                        rb = nrmp.tile([64, QSL], F32, tag="rb")
                        nc.gpsimd.partition_broadcast(rb[:], den0[:])
                        nc.vector.tensor_tensor(
                            RT8[s][64 * hh:64 * hh + 64, pr, :],
                            av[0:64, :], rb[:], ALU.mult)
                        adv(1)
                if DEBUG and s == NQS - 1:
                    for si in range(NQS):
                        nc.sync.dma_start(
                            dbg["rt"].rearrange("p k (a q) -> p k a q", a=NQS)[:, :, si, :],
                            RT8[si][:])

            def mlp_slice(s):
                q0 = s * QSL
                x2_s = x2p.tile([P, QSL // P, D], BF16, tag="x2s")
                attnT = attTp.tile([P, KD, QSL], BF16, tag="attnT")
                for mt in range(KD):
                    ps = ppp.tile([P, QSL], F32, tag="pp")
                    for j in range(KD // 2):
                        nc.tensor.matmul(
                            ps[:], wo_s[:, mt, 2 * j:2 * j + 2, :],
                            RT8[s][:, 2 * j:2 * j + 2, :],
                            start=(j == 0), stop=(j == KD // 2 - 1),
                            perf_mode=PM.DoubleRow)
                    if generic:
                        nc.vector.tensor_scalar(
                            attnT[:, mt, :], ps[:],
                            scalar1=rc1024_t[:], scalar2=bo_t[:, mt:mt + 1],
                            op0=ALU.mult, op1=ALU.add)
                    else:
                        nc.vector.tensor_single_scalar(
                            attnT[:, mt, :], ps[:], 1.0 / 1024.0, ALU.mult)
                    yield
                # x2 + LN2 stats (sqrt batched per slice)
                mvs = p4s.tile([P, QSL // P, 2], F32, tag="mvs")
                for tloc in range(QSL // P):
                    tt = s * (QSL // P) + tloc
                    x_t = p4x.tile([P, D], F32, tag="xr_t")
                    nc.sync.dma_start(x_t[:], xkv[tt * P:(tt + 1) * P, :])
                    pst = ppp.tile([P, QSL], F32, tag="pp")
                    for mt in range(KD):
                        nc.tensor.transpose(
                            pst[:].bitcast(BF16)[:, mt * P:(mt + 1) * P],
                            attnT[:, mt, tloc * P:(tloc + 1) * P], ident_b[:])
                    nc.vector.tensor_tensor(
                        x2_s[:, tloc, :], pst[:].bitcast(BF16), x_t[:], ALU.add)
                    stats = p4s.tile([P, 2, 6], F32, tag="stats2")
                    xv = x2_s[:, tloc, :].rearrange("p (a f) -> p a f", a=2)
                    for sh in range(2):
                        nc.vector.bn_stats(stats[:, sh, :], xv[:, sh, :])
                    nc.vector.bn_aggr(mvs[:, tloc, :], stats[:])
                    yield
                stds = p4s.tile([P, QSL // P], F32, tag="stds")
                nc.scalar.activation(
                    stds[:], mvs[:].rearrange("p a b -> p (a b)")[:, 1::2],
                    AF.Sqrt, bias=eps_t[:])
                nc.vector.reciprocal(stds[:], stds[:])
                for tloc in range(QSL // P):
                    tt = s * (QSL // P) + tloc
                    xn2_t = p4y.tile([P, D], BF16, tag="xn2_t")
                    if generic:
                        xn2f = p4y.tile([P, D], F32, tag="xn2f")
                        nc.vector.tensor_scalar(
                            xn2f[:], x2_s[:, tloc, :], scalar1=mvs[:, tloc, 0:1],
                            scalar2=stds[:, tloc:tloc + 1],
                            op0=ALU.subtract, op1=ALU.mult)
                        nc.gpsimd.tensor_tensor(xn2f[:], xn2f[:], g2_rep[:], ALU.mult)
                        nc.gpsimd.tensor_tensor(xn2_t[:], xn2f[:], be2_rep[:], ALU.add)
                    else:
                        nc.vector.tensor_scalar(
                            xn2_t[:], x2_s[:, tloc, :], scalar1=mvs[:, tloc, 0:1],
                            scalar2=stds[:, tloc:tloc + 1],
                            op0=ALU.subtract, op1=ALU.mult)
                    for j2 in range(KD // 2):
                        pst2 = ppp.tile([P, QSL], F32, tag="pp")
                        for hh in range(2):
                            nc.tensor.transpose(
                                pst2[:].bitcast(BF16)[:, hh * P:(hh + 1) * P],
                                xn2_t[:, (2 * j2 + hh) * P:(2 * j2 + hh + 1) * P],
                                ident_b[:])
                        nc.vector.tensor_copy(
                            xn2T[s][:, 2 * j2:2 * j2 + 2, tloc * P:(tloc + 1) * P],
                            pst2[:].bitcast(BF16)[:, 0:2 * P].rearrange(
                                "p (a m) -> p a m", a=2))
                    yield
                if DEBUG:
                    nc.sync.dma_start(
                        dbg["xn2T"].rearrange("p k (a q) -> p k a q", a=NQS)[:, :, s, :],
                        xn2T[s][:])
                    if s == 0:
                        nc.sync.dma_start(dbg["x2"][:, 0:QSL // P, :], x2_s[:])
                # h1: bf16 matmuls, bias in the psum copy, gelu deferred+batched
                h1T = h1p.tile([P, FT, QSL], BF16, tag="h1T")
                for fg in range(FT // 2):
                    w1c = wst1.tile([P, 2, 2, KD, P], FP8, tag="w1c")
                    nc.sync.dma_start(w1c[:], w1[:, 2 * fg:2 * fg + 2])
                    for f2 in range(2):
                        ft = 2 * fg + f2
                        ps = ppp.tile([P, QSL], F32, tag="pp")
                        for half in range(2):
                            for j in range(KD // 2):
                                nc.tensor.matmul(
                                    ps[:], w1c[:, f2, half, 2 * j:2 * j + 2, :],
                                    xn2T[s][:, 2 * j:2 * j + 2, :],
                                    start=(half == 0 and j == 0),
                                    stop=(half == 1 and j == KD // 2 - 1),
                                    perf_mode=PM.DoubleRow)
                        nc.vector.tensor_scalar(
                            h1T[:, ft, :], ps[:], scalar1=rc32_t[:],
                            scalar2=b1_t[:, ft:ft + 1], op0=ALU.mult, op1=ALU.add)
                        yield
                for gg in range(4):
                    nc.scalar.activation(
                        h1T[:, 8 * gg:8 * gg + 8, :],
                        h1T[:, 8 * gg:8 * gg + 8, :], AF.Gelu)
                yield
                if DEBUG and s == 0:
                    nc.sync.dma_start(dbg["h1"], h1T[:])
                # h2 + transpose + final residual + store
                outT = attTp.tile([P, KD, QSL], BF16, tag="attnT")
                for mt in range(KD):
                    ps = ppp.tile([P, QSL], F32, tag="pp")
                    for fh in range(2):
                        w2c = wst2.tile([P, FT // 2, P], BF16, tag="w2c")
                        nc.sync.dma_start(
                            w2c[:], w2[:, mt, fh * (FT // 2):(fh + 1) * (FT // 2)])
                        for fi in range(FT // 2):
                            ft = fh * (FT // 2) + fi
                            nc.tensor.matmul(
                                ps[:], w2c[:, fi, :], h1T[:, ft, :],
                                start=(ft == 0), stop=(ft == FT - 1))
                    nc.vector.tensor_scalar_add(
                        outT[:, mt, :], ps[:], b2_t[:, mt:mt + 1])
                    yield
                for tloc in range(QSL // P):
                    tt = s * (QSL // P) + tloc
                    pst = ppp.tile([P, QSL], F32, tag="pp")
                    for mt in range(KD):
                        nc.tensor.transpose(
                            pst[:].bitcast(BF16)[:, mt * P:(mt + 1) * P],
                            outT[:, mt, tloc * P:(tloc + 1) * P], ident_b[:])
                    ob = p4y.tile([P, D], F32, tag="ob")
                    nc.vector.tensor_tensor(
                        ob[:], pst[:].bitcast(BF16), x2_s[:, tloc, :], ALU.add)
                    nc.sync.dma_start(out[tt * P:(tt + 1) * P, :], ob[:])
                    yield

            gen = kproj_all()
            for s in range(NQS):
                attention_slice(s, gen)
                if gen is not None:
                    for _ in gen:
                        pass
                gen = mlp_slice(s)
            for _ in gen:
                pass

        xn2_es.close()
        x2_es.close()
        att_es.close()
        xn_es.close()
        wqk_es.close()
        es.close()

    nc.compile()
    return nc


def kernel(**inputs):
    import ml_dtypes
    E4 = ml_dtypes.float8_e4m3

    inputs = {k: np.ascontiguousarray(np.asarray(v), dtype=np.float32)
              for k, v in inputs.items()}
    generic = not (
        np.all(inputs["ln1_g"] == 1.0) and np.all(inputs["ln1_b"] == 0.0)
        and np.all(inputs["ln2_g"] == 1.0) and np.all(inputs["ln2_b"] == 0.0)
        and np.all(inputs["bq"] == 0.0) and np.all(inputs["bk"] == 0.0)
        and np.all(inputs["bv"] == 0.0) and np.all(inputs["bo"] == 0.0))
    key = ("nc", generic)
    if key not in _CACHE:
        _CACHE[key] = _build(generic=generic)
    nc = _CACHE[key]

    x = inputs["x"]

    def tile8(W, npart):
        Din, Dout = W.shape
        t = (W * 32.0).reshape(Din // P, P, npart, Dout // npart).transpose(1, 2, 0, 3)
        return np.ascontiguousarray(t.astype(E4))

    def tile8s(W, nt):
        Din, Dout = W.shape
        t = (W * 32.0).reshape(Din // P, P, nt, Dout // nt).transpose(1, 2, 0, 3)
        hi = t.astype(E4)
        lo = (t - hi.astype(np.float32)).astype(E4)
        return np.ascontiguousarray(
            np.stack([hi, lo], axis=2))  # [P, nt, 2, KD, m]

    def tileb(W, nt):
        Din, Dout = W.shape
        t = W.reshape(Din // P, P, nt, Dout // nt).transpose(1, 2, 0, 3)
        return np.ascontiguousarray(t.astype(ml_dtypes.bfloat16))

    shared = {
        "wq": tile8(inputs["Wq"], NPAIR), "wk": tile8(inputs["Wk"], NPAIR),
        "wv": tile8(inputs["Wv"], NPAIR // 2), "wo": tile8(inputs["Wo"], KD),
        "w1": tile8s(inputs["W1"], FT), "w2": tileb(inputs["W2"], KD),
        "b1v": inputs["b1"], "b2v": inputs["b2"],
    }
    if generic:
        shared.update({
            "bqv": inputs["bq"] * 32.0, "bkv": inputs["bk"] * 32.0,
            "bvv": inputs["bv"] * 32.0, "bov": inputs["bo"],
            "g1v": inputs["ln1_g"], "be1v": inputs["ln1_b"],
            "g2v": inputs["ln2_g"], "be2v": inputs["ln2_b"],
        })

    in_maps = []
    for c in range(8):
        b, half = c // 2, c % 2
        m = dict(shared)
        m["xkv"] = np.ascontiguousarray(
            np.concatenate([x[b, half * NQ:(half + 1) * NQ, :],
                            x[b, (1 - half) * NQ:(2 - half) * NQ, :]], axis=0))
        in_maps.append(m)

    res = bass_utils.run_bass_kernel_spmd(nc, in_maps, core_ids=list(range(8)))
    _CACHE["last_results"] = res
    _CACHE["nc"] = nc
    _CACHE["last_in_maps"] = in_maps

    outa = np.empty((B, S, D), dtype=np.float32)
    for c in range(8):
        b, half = c // 2, c % 2
        outa[b, half * NQ:(half + 1) * NQ, :] = res.results[c]["out"]
    return outa
